# revision 29
# baseline (speedup 1.0000x reference)
"""Trainium2 Bass kernel for nn_Attention_85057532330254.

Self-attention block (conv1x1 QKV + BatchNorm, relative-position bias,
softmax, gelu, out-projection + BatchNorm), batch-sharded across 8 cores.

Design (per core, 2 images = 2048 tokens):
 - x is PE-transposed on chip; Q^T/K^T/V^T computed directly in
   [channel, token] layout so BatchNorm stats are free-dim reductions and
   the BN affine is a per-partition scale/bias.
 - BN uses global batch stats -> two tiny AllReduces (qkv stats, z stats).
 - Softmax: exp(dots + bias) = exp(dots) * exp(bias).  exp(bias) has only
   8*63*63 distinct values (it depends on (|dx|,|dy|) only), so the host
   sends the small "fan table" V[h,u,v] = exp(E_h[|31-u|,|31-v|]/scale)
   (bf16, 64KB, packed in the sharded weight blob) and the kernel expands
   it on chip in two fan levels, positive strides only (negative DMA
   strides are silently broken on HW):
     U_fan[yj, h, (f,yi)] = V[h, f, 31-yj+yi]       (32 windowed DMAs)
     W_h[32c+yj, w]       = U_fan row slice shifted by -32c (4 bulk DMAs)
   after which every [128,1024] bias strip is a plain shifted column view
   W[:, 896-128*jc : +1024] read directly by the exp*bias multiply.  The
   values are byte-identical to precomputing exp(bias) on the host.
 - Weights travel as a 1/8-sharded packed bf16 blob (wqkv|wout|vfan) and
   are AllGathered over NeuronLink, so the host uplink carries ~0.85MB
   instead of 8x copies.
 - Scores are built transposed (sT[j,i]) so attn@V needs no transposes;
   V_aug carries a ones-column producing softmax row-sums for free; the
   attn@V accumulation is interleaved with the score pipeline so the PE
   never blocks the exp stream.
 - All 16 gelus are batched after attention (one act-table switch instead
   of two per head) with BN1's folded V affine as per-partition
   scale/bias; attention output is built transposed (g^T) so the output
   projection needs no transpose either.
 - BN2 stats via ones-column matmul reductions; second AllReduce;
   final affine applied on DVE, result (bf16) DMA'd out.

Host-side runner: the PJRT executable is jitted once and cached; later
calls only ship the (much smaller) inputs: x and the output travel in
bf16, and the donated output-seed buffer is recycled from the previous
call's output so no per-call zero upload is needed.
"""

import os

import numpy as np
import ml_dtypes

import concourse.bass as bass
import concourse.mybir as mybir
import concourse.tile as tile
from concourse import bacc
from concourse.ap import AP as BAP
from concourse.bass import ts
from concourse.masks import make_identity

F32 = mybir.dt.float32
BF16 = mybir.dt.bfloat16
AF = mybir.ActivationFunctionType
ALU = mybir.AluOpType

FMAP = 32
HEADS = 8
DK = 32
DV = 64
EPS = 1e-5
N_TOK = FMAP * FMAP            # 1024 tokens per image
DIM = 256
INNER_K = HEADS * DK           # 256
INNER_V = HEADS * DV           # 512
SCALE = DK ** -0.5
NCORES = 8
IMGS = 2                        # images per core
TOKS = IMGS * N_TOK             # 2048
NTOT = float(16 * N_TOK)        # global batch size for BN stats
FAN = 2 * FMAP - 1              # 63 distinct |delta| values
FANW = FAN * FMAP               # 2016 cols per head in U_fan
WSH_ROWS = 208                  # per-core rows of the packed weight shard
WBLOB_EL = NCORES * WSH_ROWS * DIM   # padded blob elements (425984)

_cache = {}


def _build():
    from contextlib import ExitStack

    ndev = 1 if os.environ.get("KTIME") else NCORES
    nc = bacc.Bacc(
        "TRN2", target_bir_lowering=False, debug=False, num_devices=ndev
    )
    x_d = nc.dram_tensor("x", [TOKS, DIM], BF16, kind="ExternalInput").ap()
    gb_d = nc.dram_tensor("gb", [128, 16], F32, kind="ExternalInput").ap()
    wsh_d = nc.dram_tensor("wsh", [WSH_ROWS, DIM], BF16, kind="ExternalInput").ap()
    vec2_d = nc.dram_tensor("vec2", [1, 3 * DIM], F32, kind="ExternalInput").ap()
    out_d = nc.dram_tensor("out", [TOKS, DIM], BF16, kind="ExternalOutput").ap()

    with tile.TileContext(nc) as tc, ExitStack() as es:
        _kernel_body(tc, es, x_d, wsh_d, gb_d, vec2_d, out_d)
    nc.compile()
    return nc


def _kernel_body(tc, es, x_d, wsh_d, gb_d, vec2_d, out_d):
    nc = tc.nc
    RG = [list(range(NCORES))]

    # weights arrive as a 1/8 shard of a packed bf16 blob; AllGather over
    # NeuronLink reassembles the full blob in DRAM (wfull) on every core.
    # Blob element layout: wqkv [256,1024] | wout [512,256] | vfan [8,63,63]
    dramw = es.enter_context(tc.tile_pool(name="dramw", bufs=1, space="DRAM"))
    wfull = dramw.tile([NCORES * WSH_ROWS, DIM], BF16)
    win = dramw.tile([WSH_ROWS, DIM], BF16)
    nc.sync.dma_start(win[:], wsh_d[:])
    if os.environ.get("KTIME"):
        nc.sync.dma_start(wfull[0:WSH_ROWS, :], win[:])
    else:
        nc.gpsimd.collective_compute(
            "AllGather", ALU.bypass, replica_groups=RG,
            ins=[win[:].opt()], outs=[wfull[:].opt()],
        )
    wf = wfull[:]
    OFF_WQKV, OFF_WOUT, OFF_VFAN = 0, DIM * 1024, DIM * 1024 + INNER_V * DIM

    const = es.enter_context(tc.tile_pool(name="const", bufs=1))
    ident = const.tile([128, 128], BF16)
    make_identity(nc, ident)
    gb_sb = const.tile([128, 16], F32)
    nc.sync.dma_start(gb_sb[:], gb_d[:])
    vec2_sb = const.tile([1, 3 * DIM], F32)
    nc.sync.dma_start(vec2_sb[:], vec2_d[:])
    onescol = const.tile([128, 1], F32)
    nc.gpsimd.memset(onescol[:], 1.0)

    # U_fan[yj, h, (f, yi)] = V[h, f, 31-yj+yi]: per-yj shifted windows of
    # the fan table.  Head h lives at partitions 32*(h//2)+yj, column half
    # h%2 (folded so the tile only costs 8KB/partition).  Built after the
    # phase-A x loads (see below) so it stays off the critical path.
    ufan = const.tile([128, 2 * FANW], BF16)

    def build_ufan():
        uf = ufan[:]
        for yj in range(FMAP):
            # dest partitions {yj, 32+yj, 64+yj, 96+yj} x 4032 contiguous
            # cols; source rows (h, f) merge to stride-63 runs of 32.
            dst = BAP(uf.tensor, uf.offset + yj * 2 * FANW,
                      [[2 * FANW * 32, 4], [1, 2 * FANW]])
            src_ap = BAP(wf.tensor, wf.offset + OFF_VFAN + 31 - yj,
                         [[FAN, HEADS * FAN], [1, FMAP]])
            nc.sync.dma_start(dst, src_ap)

    # persistent activations
    big = es.enter_context(tc.tile_pool(name="big", bufs=1))
    QKb = [big.tile([128, TOKS], BF16, tag=f"qkb{i}", name=f"qkb{i}") for i in range(4)]
    V_aug = big.tile([128, 16, HEADS, DV + 2], BF16, name="vaug")
    gT2 = [big.tile([128, TOKS], BF16, tag=f"gt{i}", name=f"gt{i}") for i in range(4)]
    z_sb = big.tile([128, 16 * DIM], F32, name="z_sb")
    stats_sb = const.tile([128, 16], F32)
    stats_all = const.tile([128, 16], F32)
    scale_t = const.tile([128, 8], F32)
    bias_t = const.tile([128, 8], F32)

    # ---------------- phase A/B: load x, transpose, project, stats --------
    xtp = tc.tile_pool(name="xtp", bufs=1)
    xtpool = xtp.__enter__()
    XT = [xtpool.tile([128, TOKS], BF16, tag=f"xt{i}", name=f"xt{i}") for i in range(2)]
    with (
        tc.tile_pool(name="xnat", bufs=3) as xnat_pool,
        tc.tile_pool(name="trps", bufs=4, space="PSUM") as trps,
    ):
        for t in range(16):
            xn = xnat_pool.tile([128, DIM], BF16)
            nc.sync.dma_start(xn[:], x_d[ts(t, 128), :])
            for fc in range(2):
                ps = trps.tile([128, 128], BF16)
                nc.tensor.transpose(ps[:], xn[:, ts(fc, 128)], ident[:])
                nc.vector.tensor_copy(out=XT[fc][:, ts(t, 128)], in_=ps[:])

    wq_sb = [const.tile([128, 1024], BF16, tag=f"wq{i}", name=f"wq{i}") for i in range(2)]
    for kc in range(2):
        nc.sync.dma_start(
            wq_sb[kc][:],
            BAP(wf.tensor, wf.offset + OFF_WQKV + kc * 128 * 1024,
                [[1024, 128], [1, 1024]]),
        )
    # wout stacked to match gT2 pairs: partition g*64+p64 holds row
    # (2*p2+g)*64+p64 of wout, one 256-col block per p2
    wo_sb = const.tile([128, 4 * DIM], BF16, name="wo")
    for g in range(2):
        nc.sync.dma_start(
            wo_sb[64 * g:64 * g + 64, :],
            BAP(wf.tensor, wf.offset + OFF_WOUT + g * 64 * DIM,
                [[DIM, 64], [128 * DIM, 4], [1, DIM]]),
        )
    build_ufan()

    # projections chunk-by-chunk: c8 = q0 q1 k0 k1 v0 v1 v2 v3
    with (
        tc.tile_pool(name="qkraw", bufs=1) as qkraw_pool,
        tc.tile_pool(name="scratch", bufs=1) as scratch_pool,
    ):
        qkraw = []
        with tc.tile_pool(name="projps", bufs=2, space="PSUM") as projps:
          for c8 in range(8):
            ps = projps.tile([128, TOKS], F32, tag="proj")
            for ns in range(4):
                for kc in range(2):
                    nc.tensor.matmul(
                        ps[:, ts(ns, 512)],
                        lhsT=wq_sb[kc][:, ts(c8, 128)],
                        rhs=XT[kc][:, ts(ns, 512)],
                        start=(kc == 0),
                        stop=(kc == 1),
                    )
            scr = scratch_pool.tile([128, TOKS], BF16, tag="sq")
            nc.scalar.activation(
                out=scr[:], in_=ps[:], func=AF.Square,
                accum_out=stats_sb[:, 8 + c8:9 + c8],
            )
            if c8 < 4:
                raw = qkraw_pool.tile([128, TOKS], F32, tag=f"qk{c8}")
                nc.scalar.activation(
                    out=raw[:], in_=ps[:], func=AF.Identity,
                    accum_out=stats_sb[:, c8:c8 + 1],
                )
                qkraw.append(raw)
            else:
                nc.vector.tensor_reduce(
                    out=stats_sb[:, c8:c8 + 1], in_=ps[:],
                    axis=mybir.AxisListType.X, op=ALU.add,
                )

        # V natural (for attn@V lhsT): tiles [128tok, heads, 2+64]
        with tc.tile_pool(name="vps", bufs=2, space="PSUM") as vps:
            for t in range(16):
                ps = vps.tile([128, INNER_V], F32)
                for kc in range(2):
                    nc.tensor.matmul(
                        ps[:],
                        lhsT=XT[kc][:, ts(t, 128)],
                        rhs=wq_sb[kc][:, 512:1024],
                        start=(kc == 0),
                        stop=(kc == 1),
                    )
                nc.gpsimd.memset(V_aug[:, t], 1.0)
                nc.vector.tensor_copy(
                    out=V_aug[:, t, :, 1:65],
                    in_=ps.rearrange("p (h d) -> p h d", h=HEADS),
                )

        # ---- AllReduce 1: 2048 floats of (sum, sumsq) ----
        with tc.tile_pool(name="dram1", bufs=1, space="DRAM") as dram1:
            cin = dram1.tile([128, 16], F32)
            cout = dram1.tile([128, 16], F32)
            nc.sync.dma_start(cin[:], stats_sb[:])
            if os.environ.get("KTIME"):
                nc.sync.dma_start(cout[:], cin[:])
            else:
                nc.gpsimd.collective_compute(
                    "AllReduce", ALU.add, replica_groups=RG,
                    ins=[cin[:].opt()], outs=[cout[:].opt()],
                )
            nc.sync.dma_start(stats_all[:], cout[:])

        # ---- finalize BN1 affine: scale_t/bias_t [128, 8] ----
        mean = const.tile([128, 8], F32)
        ex2 = const.tile([128, 8], F32)
        veps = const.tile([128, 8], F32)
        sq0 = const.tile([128, 8], F32)
        tmp = const.tile([128, 8], F32)
        rstd = const.tile([128, 8], F32)
        nc.vector.tensor_scalar_mul(mean[:], stats_all[:, 0:8], 1.0 / NTOT)
        nc.vector.tensor_scalar_mul(ex2[:], stats_all[:, 8:16], 1.0 / NTOT)
        # veps = ex2 - mean^2 + eps
        nc.vector.scalar_tensor_tensor(
            out=tmp[:], in0=mean[:], scalar=-1.0, in1=mean[:],
            op0=ALU.mult, op1=ALU.mult,
        )
        nc.vector.tensor_add(veps[:], ex2[:], tmp[:])
        nc.vector.tensor_scalar_add(veps[:], veps[:], EPS)
        # sqrt + one Newton step: s = 0.5*(s0 + v/s0)
        nc.scalar.sqrt(sq0[:], veps[:])
        nc.vector.reciprocal(tmp[:], sq0[:])
        nc.vector.scalar_tensor_tensor(
            out=tmp[:], in0=veps[:], scalar=1.0, in1=tmp[:],
            op0=ALU.mult, op1=ALU.mult,
        )
        nc.vector.tensor_add(tmp[:], tmp[:], sq0[:])
        nc.vector.tensor_scalar_mul(tmp[:], tmp[:], 0.5)
        nc.vector.reciprocal(rstd[:], tmp[:])
        # scale = gamma * rstd ; bias = beta - mean * scale
        nc.vector.tensor_mul(scale_t[:], gb_sb[:, 0:8], rstd[:])
        nc.vector.scalar_tensor_tensor(
            out=tmp[:], in0=mean[:], scalar=-1.0, in1=scale_t[:],
            op0=ALU.mult, op1=ALU.mult,
        )
        nc.vector.tensor_add(bias_t[:], gb_sb[:, 8:16], tmp[:])
        # fold attention 1/sqrt(dk) into q
        nc.vector.tensor_scalar_mul(scale_t[:, 0:2], scale_t[:, 0:2], SCALE)
        nc.vector.tensor_scalar_mul(bias_t[:, 0:2], bias_t[:, 0:2], SCALE)

        # normalize Q/K -> bf16 (per-partition affine on ACT)
        for c8 in range(4):
            nc.scalar.activation(
                out=QKb[c8][:], in_=qkraw[c8][:], func=AF.Identity,
                bias=bias_t[:, c8:c8 + 1], scale=scale_t[:, c8:c8 + 1],
            )

        # repack per-head V scale/bias to partition base 0: col h = head h
        sv_pk = const.tile([64, 8], F32)
        bv_pk = const.tile([64, 8], F32)
        for h in range(HEADS):
            lo = 64 * (h % 2)
            c = 4 + h // 2
            nc.sync.dma_start(sv_pk[:, h:h + 1], scale_t[lo:lo + 64, c:c + 1])
            nc.sync.dma_start(bv_pk[:, h:h + 1], bias_t[lo:lo + 64, c:c + 1])

    xtp.__exit__(None, None, None)

    # ---------------- phase C: attention ----------------------------------
    with (
        tc.tile_pool(name="bpool", bufs=2) as bpool,
        tc.tile_pool(name="stpool", bufs=16) as stpool,
        tc.tile_pool(name="expool", bufs=2) as expool,
        tc.tile_pool(name="aps", bufs=2, space="PSUM") as aps,
        tc.tile_pool(name="ops", bufs=2, space="PSUM") as ops_pool,
        tc.tile_pool(name="small", bufs=2) as small,
    ):
        for h in range(HEADS):
            qk_t = h // 4
            hp = h % 4
            # W[32c+yj, w] = ufan[hg+yj, hh*FANW + w + 96 - 32c]: four
            # partition-shifted windows of this head's fan rows (positive
            # strides, 4 bulk DMAs).  Every bias strip is then just a
            # shifted column view W[:, 896-128*jc : +1024], read directly
            # by the multiply -- no per-strip copies at all.
            W = bpool.tile([128, FANW - 96], BF16, tag="W")
            hg, hh = 32 * (h // 2), h % 2
            for c in range(4):
                s0 = hh * FANW + 96 - 32 * c
                nc.sync.dma_start(
                    W[32 * c:32 * c + 32, :],
                    ufan[hg:hg + 32, s0:s0 + FANW - 96],
                )
            sv_ap = sv_pk[:, h:h + 1]
            bv_ap = bv_pk[:, h:h + 1]
            # attn @ V_aug accumulates into outp as each score tile lands,
            # keeping PE interleaved dots/av so the exp stream never waits
            # behind a monolithic av block (rows 0..63 = dv, row 64 = the
            # ones-column rowsum)
            outps = [ops_pool.tile([128, N_TOK], F32, tag="outT",
                                   name=f"outp{h}_{i}") for i in range(IMGS)]
            for jc in range(8):
                st = stpool.tile([128, 2 * N_TOK], BF16, tag="sT")
                kpos = 32 * hp
                tp = (96, 0) if hp == 3 else None
                for img in range(IMGS):
                    dots = aps.tile([128, N_TOK], F32, tag="dots")
                    for ih in range(2):
                        nc.tensor.matmul(
                            dots[:, ts(ih, 512)],
                            lhsT=QKb[2 + qk_t][kpos:kpos + 32,
                                               img * N_TOK + jc * 128:
                                               img * N_TOK + jc * 128 + 128],
                            rhs=QKb[qk_t][kpos:kpos + 32,
                                          img * N_TOK + ih * 512:
                                          img * N_TOK + ih * 512 + 512],
                            start=True, stop=True,
                            tile_position=tp,
                        )
                    ex = expool.tile([128, N_TOK], BF16, tag="exp")
                    nc.scalar.activation(out=ex[:], in_=dots[:], func=AF.Exp)
                    nc.vector.tensor_mul(
                        st[:, ts(img, N_TOK)], ex[:],
                        W[:, 896 - 128 * jc:896 - 128 * jc + N_TOK],
                    )
                    for ih in range(2):
                        nc.tensor.matmul(
                            outps[img][0:65, ts(ih, 512)],
                            lhsT=V_aug[:, img * 8 + jc, h, 1:66],
                            rhs=st[:, img * N_TOK + ih * 512:
                                   img * N_TOK + ih * 512 + 512],
                            start=(jc == 0), stop=(jc == 7),
                            skip_group_check=True,
                        )
            for img in range(IMGS):
                outp = outps[img]
                # one copy grabs values + rowsum row; the fold/unfold DMAs
                # on gpsimd cast dtypes in flight (bf16 rowsums/recips add
                # ~0.2% to the normalization, well within budget) and the
                # all-bf16 multiply runs in DVE 4x mode
                og = small.tile([65, N_TOK], BF16, tag="og")
                nc.vector.tensor_copy(out=og[:], in_=outp[0:65, :])
                rs = small.tile([8, 128], F32, tag="rs")
                nc.gpsimd.dma_start(
                    rs[:], og[64:65, :].rearrange("o (p c) -> o p c", p=8)
                )
                rinv = small.tile([8, 128], F32, tag="rinv")
                nc.vector.reciprocal(rinv[:], rs[:])
                row = small.tile([1, N_TOK], BF16, tag="row")
                nc.gpsimd.dma_start(row[0:1, :], rinv[:])
                bc = small.tile([64, N_TOK], BF16, tag="bc")
                nc.gpsimd.partition_broadcast(bc[:], row[0:1, :])
                hb = 64 * (h % 2)
                nc.vector.tensor_mul(
                    gT2[h // 2][hb:hb + 64, ts(img, N_TOK)], og[0:64, :], bc[:])

    # ---------------- phase D: out-projection + BN2 ------------------------
    with (
        tc.tile_pool(name="zps", bufs=2, space="PSUM") as zps,
        tc.tile_pool(name="sps", bufs=1, space="PSUM") as sps,
        tc.tile_pool(name="zmisc", bufs=2) as zmisc,
        tc.tile_pool(name="dram2", bufs=1, space="DRAM") as dram2,
        tc.tile_pool(name="fin", bufs=1) as fin,
    ):
        # batched gelu (single act-table switch), in place on gT2;
        # BN1's folded V affine comes in via per-partition scale/bias.
        # The fence keeps the scheduler from interleaving these with the
        # attention exps (each interleave costs two act-table reloads).
        tc.no_sync_barrier()
        for img in range(IMGS):
            for h in range(HEADS):
                hb = 64 * (h % 2)
                g_ap = gT2[h // 2][hb:hb + 64, ts(img, N_TOK)]
                nc.scalar.activation(
                    out=g_ap, in_=g_ap, func=AF.Gelu_apprx_tanh,
                    bias=bv_pk[:, h:h + 1], scale=sv_pk[:, h:h + 1],
                )
        sums_ps = sps.tile([1, 2 * DIM], F32)
        for t in range(16):
            ps = zps.tile([128, DIM], F32, tag="z")
            for p2 in range(4):
                nc.tensor.matmul(
                    ps[:],
                    lhsT=gT2[p2][:, ts(t, 128)],
                    rhs=wo_sb[:, ts(p2, DIM)],
                    start=(p2 == 0), stop=(p2 == 3),
                )
            nc.vector.tensor_copy(out=z_sb[:, ts(t, DIM)], in_=ps[:])
            z2 = zmisc.tile([128, DIM], F32, tag="z2")
            nc.vector.tensor_mul(z2[:], z_sb[:, ts(t, DIM)], z_sb[:, ts(t, DIM)])
            nc.tensor.matmul(
                sums_ps[0:1, 0:DIM], lhsT=onescol[:], rhs=z_sb[:, ts(t, DIM)],
                start=(t == 0), stop=(t == 15), skip_group_check=True,
            )
            nc.tensor.matmul(
                sums_ps[0:1, DIM:2 * DIM], lhsT=onescol[:], rhs=z2[:],
                start=(t == 0), stop=(t == 15), skip_group_check=True,
            )
        st2 = fin.tile([1, 2 * DIM], F32)
        nc.vector.tensor_copy(out=st2[:], in_=sums_ps[:])
        cin = dram2.tile([1, 2 * DIM], F32)
        cout = dram2.tile([1, 2 * DIM], F32)
        nc.sync.dma_start(cin[:], st2[:])
        if os.environ.get("KTIME"):
            nc.sync.dma_start(cout[:], cin[:])
        else:
            nc.gpsimd.collective_compute(
                "AllReduce", ALU.add, replica_groups=RG,
                ins=[cin[:].opt()], outs=[cout[:].opt()],
            )
        st2a = fin.tile([1, 2 * DIM], F32)
        nc.sync.dma_start(st2a[:], cout[:])

        # finalize BN2 on [1, 256] rows.  z_true = z_raw + b_out
        mean = fin.tile([1, DIM], F32)
        ex2 = fin.tile([1, DIM], F32)
        veps = fin.tile([1, DIM], F32)
        sq0 = fin.tile([1, DIM], F32)
        tmp = fin.tile([1, DIM], F32)
        s2 = fin.tile([1, DIM], F32)
        b2f = fin.tile([1, DIM], F32)
        b_out_row = vec2_sb[0:1, 0:DIM]
        go_row = vec2_sb[0:1, DIM:2 * DIM]
        bo_row = vec2_sb[0:1, 2 * DIM:3 * DIM]
        nc.vector.tensor_scalar_mul(mean[:], st2a[0:1, 0:DIM], 1.0 / NTOT)
        nc.vector.tensor_scalar_mul(ex2[:], st2a[0:1, DIM:2 * DIM], 1.0 / NTOT)
        # ex2_true = ex2 + 2*mean*b_out + b_out^2 ; m_true = mean + b_out
        nc.vector.scalar_tensor_tensor(
            out=tmp[:], in0=mean[:], scalar=2.0, in1=b_out_row,
            op0=ALU.mult, op1=ALU.mult,
        )
        nc.vector.tensor_add(ex2[:], ex2[:], tmp[:])
        nc.vector.tensor_mul(tmp[:], b_out_row, b_out_row)
        nc.vector.tensor_add(ex2[:], ex2[:], tmp[:])
        m_true = fin.tile([1, DIM], F32)
        nc.vector.tensor_add(m_true[:], mean[:], b_out_row)
        nc.vector.scalar_tensor_tensor(
            out=tmp[:], in0=m_true[:], scalar=-1.0, in1=m_true[:],
            op0=ALU.mult, op1=ALU.mult,
        )
        nc.vector.tensor_add(veps[:], ex2[:], tmp[:])
        nc.vector.tensor_scalar_add(veps[:], veps[:], EPS)
        nc.scalar.sqrt(sq0[:], veps[:])
        nc.vector.reciprocal(tmp[:], sq0[:])
        nc.vector.scalar_tensor_tensor(
            out=tmp[:], in0=veps[:], scalar=1.0, in1=tmp[:],
            op0=ALU.mult, op1=ALU.mult,
        )
        nc.vector.tensor_add(tmp[:], tmp[:], sq0[:])
        nc.vector.tensor_scalar_mul(tmp[:], tmp[:], 0.5)
        nc.vector.reciprocal(tmp[:], tmp[:])        # rstd2
        nc.vector.tensor_mul(s2[:], go_row, tmp[:])
        # bias2_final = bo - mean_raw * s2
        nc.vector.scalar_tensor_tensor(
            out=tmp[:], in0=mean[:], scalar=-1.0, in1=s2[:],
            op0=ALU.mult, op1=ALU.mult,
        )
        nc.vector.tensor_add(b2f[:], bo_row, tmp[:])
        # narrow [128, 256] broadcasts; per-chunk affine + store
        bcs2 = fin.tile([128, DIM], F32)
        bcb2 = fin.tile([128, DIM], F32)
        nc.gpsimd.partition_broadcast(bcs2[:], s2[:])
        nc.gpsimd.partition_broadcast(bcb2[:], b2f[:])
        zo = fin.tile([128, 16 * DIM], BF16)
        for t in range(16):
            nc.vector.tensor_mul(
                z_sb[:, ts(t, DIM)], z_sb[:, ts(t, DIM)], bcs2[:])
            nc.vector.tensor_add(zo[:, ts(t, DIM)], z_sb[:, ts(t, DIM)], bcb2[:])
        nc.sync.dma_start(
            out_d.rearrange("(t p) c -> p t c", p=128), zo.rearrange("p (t c) -> p t c", t=16)
        )


def _make_runner():
    """Build nc once, jit the PJRT executable once, return a reusable callable."""
    import jax
    import jax.numpy as jnp
    from jax.sharding import Mesh, PartitionSpec, NamedSharding
    from jax.experimental.shard_map import shard_map
    from concourse import bass2jax

    nc = _build()
    bass2jax.install_neuronx_cc_hook()
    partition_name = nc.partition_id_tensor.name if nc.partition_id_tensor else None
    in_names, out_names, out_avals = [], [], []
    for alloc in nc.m.functions[0].allocations:
        if not isinstance(alloc, mybir.MemoryLocationSet):
            continue
        name = alloc.memorylocations[0].name
        if alloc.kind == "ExternalInput":
            if name != partition_name:
                in_names.append(name)
        elif alloc.kind == "ExternalOutput":
            out_names.append(name)
            out_avals.append(
                jax.core.ShapedArray(tuple(alloc.tensor_shape), mybir.dt.np(alloc.dtype))
            )
    n_params = len(in_names)
    n_outs = len(out_names)
    all_in_names = list(in_names) + out_names
    if partition_name:
        all_in_names.append(partition_name)

    def _body(*args):
        ops = list(args)
        if partition_name is not None:
            ops.append(bass2jax.partition_id_tensor())
        return tuple(bass2jax._bass_exec_p.bind(
            *ops, out_avals=tuple(out_avals), in_names=tuple(all_in_names),
            out_names=tuple(out_names), lowering_input_output_aliases=(),
            sim_require_finite=True, sim_require_nnan=True, nc=nc))

    devices = jax.devices()[:NCORES]
    mesh = Mesh(np.asarray(devices), ("core",))
    shC = NamedSharding(mesh, PartitionSpec("core"))
    donate = tuple(range(n_params, n_params + n_outs))
    sharded = jax.jit(
        shard_map(
            _body, mesh=mesh,
            in_specs=(PartitionSpec("core"),) * (n_params + n_outs),
            out_specs=(PartitionSpec("core"),) * n_outs, check_rep=False),
        donate_argnums=donate, keep_unused=True)
    zshapes = [(NCORES * av.shape[0], *av.shape[1:]) for av in out_avals]
    zdtypes = [av.dtype for av in out_avals]
    mkzeros = jax.jit(
        lambda: tuple(jnp.zeros(s, d) for s, d in zip(zshapes, zdtypes)),
        out_shardings=tuple(shC for _ in zshapes))

    state = {"donors": None}

    def run(in_concat: dict):
        args = [in_concat[nm] for nm in in_names]
        donors = state["donors"] or list(mkzeros())
        out_arrs = sharded(*args, *donors)
        res = [np.asarray(a) for a in out_arrs]
        state["donors"] = list(out_arrs)
        return dict(zip(out_names, res))

    return run


def kernel(**inputs):
    f = np.float32
    x = np.asarray(inputs["x"], f)
    wq, wk, wv = (np.asarray(inputs[k], f) for k in ("wq", "wk", "wv"))
    pos_emb = np.asarray(inputs["pos_emb"], f)
    w_out = np.asarray(inputs["w_out"], f)

    # fan table: V[h,u,v] = exp(E_h[|31-u|, |31-v|] / scale), E_h = pos_emb
    # reshaped per head over (|dx|, |dy|).  Same values (and same bf16
    # rounding) as precomputing exp(bias) for the full [n,n] matrix.
    E = np.exp(pos_emb.T.reshape(HEADS, FMAP, FMAP) / SCALE)
    fi = np.abs(31 - np.arange(FAN))
    V = np.ascontiguousarray(E[:, fi[:, None], fi[None, :]]).astype(ml_dtypes.bfloat16)

    wqkv = np.concatenate([wq, wk, wv], axis=1)      # [256, 1024]
    # gb: col 0-7 gammas, 8-15 betas, chunk order q0 q1 k0 k1 v0..v3
    gcat = np.concatenate(
        [np.asarray(inputs["gq"], f), np.asarray(inputs["gk"], f),
         np.asarray(inputs["gv"], f)]
    ).reshape(8, 128).T
    bcat = np.concatenate(
        [np.asarray(inputs["bq"], f), np.asarray(inputs["bk"], f),
         np.asarray(inputs["bv"], f)]
    ).reshape(8, 128).T
    gb = np.concatenate([gcat, bcat], axis=1).copy()  # [128, 16]
    vec2 = np.concatenate(
        [np.asarray(inputs["b_out"], f), np.asarray(inputs["go"], f),
         np.asarray(inputs["bo"], f)]
    )[None, :].copy()                                 # [1, 768]

    if "run" not in _cache:
        _cache["run"] = _make_runner()
    run = _cache["run"]

    # concat-along-axis-0 layout expected by the sharded executable:
    # x is batch-sharded (its reshape is already the concat); the packed
    # weight blob is sharded 1/8 per core and AllGathered on device; the
    # tiny gb/vec2 stay replicated.
    blob = np.zeros(WBLOB_EL, ml_dtypes.bfloat16)
    n_wqkv = DIM * 1024
    n_wout = INNER_V * DIM
    blob[0:n_wqkv] = wqkv.astype(ml_dtypes.bfloat16).ravel()
    blob[n_wqkv:n_wqkv + n_wout] = w_out.astype(ml_dtypes.bfloat16).ravel()
    blob[n_wqkv + n_wout:n_wqkv + n_wout + V.size] = V.ravel()
    in_concat = {
        "x": np.ascontiguousarray(x.reshape(16 * N_TOK, DIM)).astype(ml_dtypes.bfloat16),
        "gb": np.tile(gb, (NCORES, 1)),
        "wsh": blob.reshape(NCORES * WSH_ROWS, DIM),
        "vec2": np.tile(vec2, (NCORES, 1)),
    }
    res = run(in_concat)
    out = res["out"].astype(np.float32)
    return out.reshape(16, FMAP, FMAP, DIM)


if __name__ == "__main__":
    if os.environ.get("BUILD_ONLY"):
        _build()
        print("BUILD OK")
